# revision 39
# baseline (speedup 1.0000x reference)
"""Trainium2 Bass kernel for nn_AttentionLayer (b=4, l=s=2048, D=64, H=8, hd=8).

Sharding: 8 cores = 4 batches x 2 head-quads (4 heads each). Every core runs
the identical SPMD program over its batch's full causal triangle for its 4
heads; the host sums the two per-batch output-projection partials (standard
tensor-parallel reduction over heads).

Per-core dataflow (all-bf16 matmuls, fp32 PSUM accumulate):
  qT/kT = W_pad @ x_augT   (bf16; heads padded to 32-partition row groups)
  scoresT[s,l] per head via bf16 row-tiled matmuls (K=8, tile_position=(32r,0))
  exp: per half-span greedy-balanced between ACT (exact table exp) and DVE
  (1-pass Schraudolph: out_int16 = round(A*x + B) bit-cast as bf16 ~= exp(x),
  ~1.5% rms err); causal diagonal masked by a tri multiply on DVE
  AV^T + denominator via bf16 col-tiled matmuls with [V|1] stationary operand,
  three spans software-pipelined behind the exp
  normalize with reciprocal_approx_fast + bf16 PE broadcast matmul
  y^T = Wo_pad @ av_norm (bf16) + bo/2 via ACT Copy-with-bias, DMA'd out
"""

import os
import sys

for _p in ("/opt/trn_rl_repo", "/root/.axon_site/_ro/trn_rl_repo"):
    if os.path.isdir(_p) and _p not in sys.path:
        sys.path.append(_p)

import numpy as np

H = 8
D = 64
HD = 8
B = 4
L = 2048
SCALE = 1.0 / np.sqrt(np.float32(HD))

NT = L // 128   # 16 s-tiles of 128
NG = 4          # groups of 512 query columns

_CACHE = {}
LAST_EXEC_NS = None

# Schraudolph bf16 exp: bits = round(A*x + B); bitcast(int16) ~= exp(x)
EXP_A = float(2.0 ** 7 / np.log(2.0))      # 184.6650
EXP_B = float(127.0 * 2.0 ** 7 - 4.7)      # centered offset


def _build(causal: bool):
    import concourse.bacc as bacc
    import concourse.tile as tile
    import concourse.mybir as mybir

    f32 = mybir.dt.float32
    bf16 = mybir.dt.bfloat16
    i16 = mybir.dt.int16
    AF = mybir.ActivationFunctionType
    OP = mybir.AluOpType

    nc = bacc.Bacc("TRN2", target_bir_lowering=False, debug=False, num_devices=8)

    xq_d = nc.dram_tensor("xq", [65, L], bf16, kind="ExternalInput").ap()
    xk_d = nc.dram_tensor("xk", [65, L], bf16, kind="ExternalInput").ap()
    xv_d = nc.dram_tensor("xv", [65, L], bf16, kind="ExternalInput").ap()
    # blobA: [65, 292] = wq[0:128] | wk[128:256] | wv[256:292]
    wa_d = nc.dram_tensor("wa", [65, 292], bf16, kind="ExternalInput").ap()
    # blobB: [128, 193] = wo[0:64] | sel[64:192] | bo2 col [0:64, 192]
    wb_d = nc.dram_tensor("wb", [128, 193], bf16, kind="ExternalInput").ap()
    tri_d = nc.dram_tensor("tri", [128, 512], bf16, kind="ExternalInput").ap()
    y_d = nc.dram_tensor("y", [64, L], f32, kind="ExternalOutput").ap()

    from contextlib import ExitStack

    # ---- build-time ACT/DVE exp assignment (greedy balance of ns totals) ----
    def act_cost(free):
        return (free + 352) / 1.2

    def dve_cost(free):
        return (free + 120) / 0.96

    assign = {}
    act_tot = [1283.0 + 430.0]  # table load + warm
    dve_tot = [0.0]

    def plan_group(g):
        if g % 2 == 0:
            # proj copies split across both engines (k/q halves + v)
            act_tot[0] += 2 * act_cost(512) + act_cost(288)
            dve_tot[0] += 2 * dve_cost(512) + dve_cost(288)
        n_s = 4 * g + 4 if causal else NT
        if g == 0:
            dve_tot[0] += 484 + 594  # av memset + dummy-warm memset
        for t in range(n_s):
            col_off = max(0, (t - 4 * g) * 128) if causal else 0
            npr = 512 - col_off
            if causal and t >= 4 * g:
                dve_tot[0] += (151 + 256) / 0.96  # tri mask (bf16 2x)
            for p_ in range(2):
                fa, fd = act_cost(2 * npr), dve_cost(2 * npr)
                if act_tot[0] + fa <= dve_tot[0] + fd:
                    assign[(g, t, p_)] = "ACT"
                    act_tot[0] += fa
                else:
                    assign[(g, t, p_)] = "DVE"
                    dve_tot[0] += fd
        # epilogue: av_sb copy on ACT; rcp + cast + avn + y bias on DVE
        act_tot[0] += act_cost(512)
        dve_tot[0] += (58 + 512) / 0.96 + (58 + 256) / 0.96 \
            + (151 + 512) / 0.96 + dve_cost(512)

    for g in range(NG):
        plan_group(g)

    with tile.TileContext(nc) as tc, ExitStack() as es:
        singles = es.enter_context(tc.tile_pool(name="singles", bufs=1))
        persist = es.enter_context(tc.tile_pool(name="persist", bufs=1))
        sc_ps = es.enter_context(tc.tile_pool(name="sc_ps", bufs=3, space="PSUM"))
        av_ps_pool = es.enter_context(tc.tile_pool(name="av_ps", bufs=1, space="PSUM"))
        misc_ps = es.enter_context(tc.tile_pool(name="misc_ps", bufs=1, space="PSUM"))
        attn_pool = es.enter_context(tc.tile_pool(name="attn", bufs=8))
        small = es.enter_context(tc.tile_pool(name="small", bufs=2))

        # ---- load inputs ----
        xq = singles.tile([65, L], bf16, tag="xq")
        xk = singles.tile([65, L], bf16, tag="xk")
        xv = singles.tile([65, L], bf16, tag="xv")
        wa = singles.tile([65, 292], bf16, tag="wa")
        wb = singles.tile([128, 193], bf16, tag="wb")
        tri = singles.tile([128, 512], bf16, tag="tri")

        wq = wa[:, 0:128]
        wk = wa[:, 128:256]
        wv = wa[:, 256:292]
        wo = wb[:, 0:64]
        sel = wb[:, 64:192]

        # spread input DMA: weights on scalar (idle early), xk/xv on sync,
        # xq on gpsimd, so the wk+xk0 / xq0 critical chains run in parallel
        s0, s1 = slice(0, 1024), slice(1024, 2048)
        nc.scalar.dma_start(out=wa[:], in_=wa_d[:])
        nc.sync.dma_start(out=xk[:, s0], in_=xk_d[:, s0])
        nc.gpsimd.dma_start(out=xq[:, s0], in_=xq_d[:, s0])
        nc.scalar.dma_start(out=tri[:], in_=tri_d[:])
        nc.sync.dma_start(out=xv[:, s0], in_=xv_d[:, s0])
        nc.gpsimd.dma_start(out=wb[:], in_=wb_d[:])
        nc.sync.dma_start(out=xk[:, s1], in_=xk_d[:, s1])
        nc.gpsimd.dma_start(out=xq[:, s1], in_=xq_d[:, s1])
        nc.sync.dma_start(out=xv[:, s1], in_=xv_d[:, s1])

        # preload the ACT exp table during the rest of the DMA
        warm = singles.tile([1, 292], f32, tag="warm")
        nc.scalar.activation(out=warm[:], in_=wa[:1, :], func=AF.Exp)
        bo_col = singles.tile([64, 1], f32, tag="bo_col")

        # ---- persistent projected tensors ----
        qT = persist.tile([128, L], bf16, tag="qT")
        kT = persist.tile([128, L], bf16, tag="kT")
        v_aug = persist.tile([128, 16 * 36], bf16, tag="v_aug")

        def proj_piece(jh, piece):
            # one fine projection piece (1 matmul + 1 copy) staged through the
            # misc PSUM bank so the sc pool stays dedicated to span tiles;
            # copies alternate ACT/DVE so no engine eats a burst
            act = piece in ("k0", "q1", "v0")
            pj = misc_ps.tile([128, 512], f32, tag="misc")
            if piece[0] in "kq":
                m = int(piece[1])
                w_, dst, src_x = (wk, kT, xk) if piece[0] == "k" else (wq, qT, xq)
                s2 = slice(1024 * jh + 512 * m, 1024 * jh + 512 * (m + 1))
                nc.tensor.matmul(out=pj[:], lhsT=w_, rhs=src_x[:, s2],
                                 start=True, stop=True)
                if jh == 0 and m == 0:
                    # startup critical path: split across both engines
                    cut = 128 if piece[0] == "k" else 256
                    nc.scalar.copy(dst[:, 0:cut], pj[:, 0:cut])
                    nc.vector.tensor_copy(dst[:, cut:512], pj[:, cut:512])
                elif act:
                    nc.scalar.copy(dst[:, s2], pj[:])
                else:
                    nc.vector.tensor_copy(dst[:, s2], pj[:])
            else:
                half = int(piece[1])  # "v0" / "v1"
                for tt in range(4):
                    t = 8 * jh + 4 * half + tt
                    nc.tensor.matmul(out=pj[:, 128 * tt:128 * tt + 36],
                                     lhsT=xv[:, 128 * t:128 * (t + 1)], rhs=wv,
                                     start=True, stop=True)
                dst = v_aug.rearrange("p (c n) -> p c n", n=36)[
                    :, 8 * jh + 4 * half:8 * jh + 4 * half + 4, :]
                src = pj.rearrange("p (c n) -> p c n", n=128)[:, :, :36]
                if act:
                    nc.scalar.copy(dst, src)
                else:
                    nc.vector.tensor_copy(dst, src)

        # ---- attention ----
        def make_epilogue(g, av):
            # normalize + output projection; nbl column blocks pipelined so
            # the last group's chain hides behind its trailing spans
            def run(nbl):
                w = 512 // nbl
                y_sb = small.tile([64, 512], f32, tag="y_sb")
                av_sb = small.tile([128, 512], f32, tag="av_sb")
                rcp = small.tile([128, 512], f32, tag="rcp")
                rcp_bf = small.tile([128, 512], bf16, tag="rcp_bf")
                avn = small.tile([128, 512], bf16, tag="avn")
                for b_ in range(nbl):
                    cs = slice(w * b_, w * (b_ + 1))
                    if b_ % 2 == 0:
                        nc.scalar.copy(av_sb[:, cs], av[:, cs])
                    else:
                        nc.vector.tensor_copy(av_sb[:, cs], av[:, cs])
                    nc.vector.reciprocal_approx_fast(out=rcp[:, cs], in_=av_sb[:, cs])
                    nc.vector.tensor_copy(rcp_bf[:, cs], rcp[:, cs])
                    bc = misc_ps.tile([128, 512], f32, tag="misc")
                    nc.tensor.matmul(out=bc[:, cs], lhsT=sel, rhs=rcp_bf[:, cs],
                                     start=True, stop=True)
                    nc.vector.tensor_mul(avn[:, cs], av_sb[:, cs], bc[:, cs])
                    yp = misc_ps.tile([64, 512], f32, tag="misc")
                    nc.tensor.matmul(out=yp[:, cs], lhsT=wo, rhs=avn[:, cs],
                                     start=True, stop=True)
                    nc.vector.tensor_scalar(out=y_sb[:, cs], in0=yp[:, cs],
                                            scalar1=bo_col[:],
                                            scalar2=None, op0=OP.add)
                nc.sync.dma_start(out=y_d[:, 512 * g:512 * (g + 1)], in_=y_sb[:])
            return run

        # PE warm-up: dummy matmuls fill the input-DMA wait so HAM reaches
        # 2.4 GHz before the first real matmul (results overwritten/ignored);
        # alternating output addresses avoid WAW drain serialization.
        # The memset must be the FIRST vector-queue op (no DMA dependency)
        # or the dummies start late behind DMA-gated copies.
        dummy_in = singles.tile([128, 256], bf16, tag="dummy_in")
        nc.vector.memset(dummy_in[:], 1.0)
        dmy = misc_ps.tile([128, 512], f32, tag="misc")
        for i in range(14):
            nc.tensor.matmul(out=dmy[:, 256 * (i % 2):256 * (i % 2) + 256],
                             lhsT=dummy_in[:, :128],
                             rhs=dummy_in[:], start=True, stop=True)

        pending_epi = None
        for g in range(NG):
            if g == 0:
                proj_piece(0, "k0")
                proj_piece(0, "q0")
            n_s = 4 * g + 4 if causal else NT
            # 1.0 (not 0) so junk rows stay finite through reciprocal below;
            # rows 9..31 of each 32-group persist across groups (PE never
            # writes them), so one memset at kernel start suffices
            av = av_ps_pool.tile([128, 512], f32, tag="av")
            if g == 0:
                nc.vector.memset(av[:], 1.0)
            pending_av = []

            def emit_av(t, col_off, attnT, g=g, n_s=n_s, av=av):
                for c in range(4):
                    nc.tensor.matmul(
                        out=av[32 * c:32 * c + 9, col_off:512],
                        lhsT=v_aug[:, 36 * t + 9 * c:36 * t + 9 * c + 9],
                        rhs=attnT[:, 512 * c + col_off:512 * (c + 1)],
                        start=(t == 0), stop=(t == n_s - 1),
                        tile_position=(0, 32 * c))

            for t in range(n_s):
                if len(pending_av) > 4:
                    # emit AV in batches of two spans (8 col-tiled MMs): AV
                    # alone streams ~2x faster than when interleaved with
                    # score MMs, so fewer SC/AV boundaries = less contention
                    emit_av(*pending_av.pop(0))
                    emit_av(*pending_av.pop(0))
                col_off = max(0, (t - 4 * g) * 128) if causal else 0
                npr = 512 - col_off
                scA = sc_ps.tile([128, 1024], f32, tag="sc")
                scB = sc_ps.tile([128, 1024], f32, tag="sc")
                for r in range(4):
                    sc = scA if r < 2 else scB
                    c0 = 512 * (r % 2) + col_off
                    nc.tensor.matmul(
                        out=sc[:, c0:c0 + npr],
                        lhsT=kT[32 * r:32 * r + 8, 128 * t:128 * (t + 1)],
                        rhs=qT[32 * r:32 * r + 8, 512 * g + col_off:512 * (g + 1)],
                        start=True, stop=True, tile_position=(32 * r, 0))
                attnT = attn_pool.tile([128, 2048], bf16, tag="attnT")
                a4 = attnT.rearrange("p (h n) -> p h n", h=4)
                for p_ in range(2):
                    sc = scA if p_ == 0 else scB
                    s2 = sc.rearrange("p (h n) -> p h n", h=2)
                    if assign[(g, t, p_)] == "ACT":
                        nc.scalar.activation(
                            out=a4[:, 2 * p_:2 * p_ + 2, col_off:512],
                            in_=s2[:, :, col_off:512],
                            func=AF.Exp)
                    else:
                        nc.vector.tensor_scalar(
                            out=a4[:, 2 * p_:2 * p_ + 2, col_off:512].bitcast(i16),
                            in0=s2[:, :, col_off:512],
                            scalar1=EXP_A, scalar2=EXP_B,
                            op0=OP.mult, op1=OP.add)
                if causal and t >= 4 * g:
                    t4 = tri.rearrange("p (h n) -> p h n", h=4)
                    nc.vector.tensor_mul(
                        a4[:, :, col_off:col_off + 128],
                        a4[:, :, col_off:col_off + 128],
                        t4[:, :, :])
                pending_av.append((t, col_off, attnT))
                # previous group's deferred epilogue + next projections slot in
                # behind this group's first spans (engines have slack here)
                if t == 1 and pending_epi is not None:
                    pending_epi(1)
                    pending_epi = None
                if g == 0 and 0 <= t <= 2:
                    proj_piece(0, ("v0", "q1", "k1")[t])
                if g == 0 and t == 1:
                    nc.vector.tensor_copy(bo_col[:], wb[0:64, 192:193])
                if g == 1 and t == 0:
                    proj_piece(0, "v1")
                if g == 1 and 2 <= t <= 7:
                    proj_piece(1, ("q0", "k0", "q1", "k1", "v0", "v1")[t - 2])
            for args in pending_av:
                emit_av(*args)
            pending_epi = make_epilogue(g, av)
        # last group's epilogue: 4 pipelined column blocks
        pending_epi(4 if causal else 1)

    nc.compile()
    return nc


def _prep_inputs(queries, keys, values, Wq, bq, Wk, bk, Wv, bv, Wo, bo):
    """Build the 8 per-core input maps (host-side layout/sharding only)."""
    import ml_dtypes
    bf = ml_dtypes.bfloat16
    ones = np.ones((1, L), np.float32)

    def aug_t(x_b):  # [L, 64] -> [65, L] bf16
        return np.ascontiguousarray(np.vstack([x_b.T, ones]).astype(bf))

    # padded projection weights per quad: col 32r+d <- head (4Q+r) dim d
    def w_pad(W, b, quad, scale=1.0):
        out = np.zeros((65, 128), np.float32)
        for r in range(4):
            ch = 8 * (4 * quad + r)
            out[:64, 32 * r:32 * r + 8] = W[ch:ch + 8, :].T * scale
            out[64, 32 * r:32 * r + 8] = b[ch:ch + 8] * scale
        return out

    def wv_aug(quad):  # [65, 36]: col 9c+e <- head (4Q+c) dim e; col 9c+8 = e64
        out = np.zeros((65, 36), np.float32)
        for c in range(4):
            ch = 8 * (4 * quad + c)
            out[:64, 9 * c:9 * c + 8] = Wv[ch:ch + 8, :].T
            out[64, 9 * c:9 * c + 8] = bv[ch:ch + 8]
            out[64, 9 * c + 8] = 1.0
        return out

    def wo_pad(quad):  # [128, 64]: row 32c+d -> Wo[:, 8(4Q+c)+d]
        out = np.zeros((128, 64), np.float32)
        for c in range(4):
            ch = 8 * (4 * quad + c)
            out[32 * c:32 * c + 8, :] = Wo[:, ch:ch + 8].T
        return out

    tri = (np.arange(128)[:, None] <= np.arange(128)[None, :]).astype(np.float32)
    tri4 = np.ascontiguousarray(np.tile(tri, (1, 4)).astype(bf))
    sel = np.zeros((128, 128), np.float32)
    for c in range(4):
        sel[32 * c + 8, 32 * c:32 * c + 9] = 1.0
    bo2 = (bo.astype(np.float32) / 2.0).reshape(64)

    wa_cache, wb_cache = {}, {}
    for qd in range(2):
        wa = np.zeros((65, 292), np.float32)
        wa[:, 0:128] = w_pad(Wq, bq, qd, scale=float(SCALE))
        wa[:, 128:256] = w_pad(Wk, bk, qd)
        wa[:, 256:292] = wv_aug(qd)
        wb = np.zeros((128, 193), np.float32)
        wb[:, 0:64] = wo_pad(qd)
        wb[:, 64:192] = sel
        wb[0:64, 192] = bo2
        wa_cache[qd] = np.ascontiguousarray(wa.astype(bf))
        wb_cache[qd] = np.ascontiguousarray(wb.astype(bf))

    in_maps = []
    for c in range(8):
        b, qd = c // 2, c % 2
        in_maps.append(dict(
            xq=aug_t(np.asarray(queries[b])),
            xk=aug_t(np.asarray(keys[b])),
            xv=aug_t(np.asarray(values[b])),
            wa=wa_cache[qd], wb=wb_cache[qd], tri=tri4,
        ))
    return in_maps


def _install_trace_hook():
    import contextlib
    import ctypes
    import types

    name = "antenv.axon_hooks"
    if name in sys.modules:
        return
    so_path = "/opt/axon/libaxon_pjrt.so"
    if not os.path.exists(so_path):
        return
    lib = ctypes.CDLL(so_path)
    if not hasattr(lib, "axon_start_nrt_profile"):
        return
    lib.axon_start_nrt_profile.argtypes = [ctypes.POINTER(ctypes.c_int64), ctypes.c_size_t]
    lib.axon_start_nrt_profile.restype = ctypes.c_int64
    lib.axon_stop_nrt_profile.argtypes = [ctypes.c_char_p]
    lib.axon_stop_nrt_profile.restype = ctypes.c_int64

    @contextlib.contextmanager
    def _hook(output_dir, device_ids):
        import jax
        jax.devices()
        if device_ids:
            ids = (ctypes.c_int64 * len(device_ids))(*device_ids)
            rc = lib.axon_start_nrt_profile(ids, len(device_ids))
        else:
            rc = lib.axon_start_nrt_profile(None, 0)
        if rc != 0:
            raise RuntimeError(f"axon_start_nrt_profile rc={rc}")
        try:
            yield
        finally:
            n = lib.axon_stop_nrt_profile(str(output_dir).encode())
            print(f"profile: {n} file(s) in {output_dir}", file=sys.stderr)

    mod = types.ModuleType(name)
    mod._hook = _hook
    mod.set_axon_ntff_profile_hook = lambda h: setattr(mod, "_hook", h)
    mod.get_axon_ntff_profile_hook = lambda: mod._hook
    sys.modules[name] = mod


def kernel(queries, keys, values, attention_mask, Wq, bq, Wk, bk, Wv, bv, Wo, bo):
    global LAST_EXEC_NS
    from concourse.bass_utils import run_bass_kernel_spmd

    causal = bool(int(np.asarray(attention_mask)))
    if causal not in _CACHE:
        _CACHE[causal] = _build(causal)
    nc = _CACHE[causal]

    in_maps = _prep_inputs(queries, keys, values, Wq, bq, Wk, bk, Wv, bv, Wo, bo)

    trace = os.environ.get("KERNEL_TRACE", "") == "1"
    kwargs = {}
    if trace:
        _install_trace_hook()
        kwargs = dict(trace=True, tmpdir=os.environ.get("KERNEL_TRACE_DIR") or None)
    res = run_bass_kernel_spmd(nc, in_maps, core_ids=list(range(8)), **kwargs)
    LAST_EXEC_NS = res.exec_time_ns

    out = np.empty((B, L, D), np.float32)
    for b in range(B):
        out[b] = (res.results[2 * b]["y"] + res.results[2 * b + 1]["y"]).T
    return out


# revision 41
# speedup vs baseline: 1.2239x; 1.2239x over previous
"""Trainium2 Bass kernel for nn_AttentionLayer (b=4, l=s=2048, D=64, H=8, hd=8).

Sharding: 8 cores = 4 batches x 2 head-quads (4 heads each). Every core runs
the identical SPMD program over its batch's full causal triangle for its 4
heads; the host sums the two per-batch output-projection partials (standard
tensor-parallel reduction over heads).

Per-core dataflow (all-bf16 matmuls, fp32 PSUM accumulate):
  qT/kT = W_pad @ x_augT   (bf16; heads padded to 32-partition row groups)
  scoresT[s,l] per head via bf16 row-tiled matmuls (K=8, tile_position=(32r,0))
  exp: per half-span greedy-balanced between ACT (exact table exp) and DVE
  (1-pass Schraudolph: out_int16 = round(A*x + B) bit-cast as bf16 ~= exp(x),
  ~1.5% rms err); causal diagonal masked by a tri multiply on DVE
  AV^T + denominator via bf16 col-tiled matmuls with [V|1] stationary operand,
  three spans software-pipelined behind the exp
  normalize with reciprocal_approx_fast + bf16 PE broadcast matmul
  y^T = Wo_pad @ av_norm (bf16) + bo/2 via ACT Copy-with-bias, DMA'd out
"""

import os
import sys

for _p in ("/opt/trn_rl_repo", "/root/.axon_site/_ro/trn_rl_repo"):
    if os.path.isdir(_p) and _p not in sys.path:
        sys.path.append(_p)

import numpy as np

H = 8
D = 64
HD = 8
B = 4
L = 2048
SCALE = 1.0 / np.sqrt(np.float32(HD))

NT = L // 128   # 16 s-tiles of 128
NG = 4          # groups of 512 query columns

_CACHE = {}
LAST_EXEC_NS = None

# Schraudolph bf16 exp: bits = round(A*x + B); bitcast(int16) ~= exp(x)
EXP_A = float(2.0 ** 7 / np.log(2.0))      # 184.6650
EXP_B = float(127.0 * 2.0 ** 7 - 4.7)      # centered offset


def _build(causal: bool):
    import concourse.bacc as bacc
    import concourse.tile as tile
    import concourse.mybir as mybir

    f32 = mybir.dt.float32
    bf16 = mybir.dt.bfloat16
    i16 = mybir.dt.int16
    AF = mybir.ActivationFunctionType
    OP = mybir.AluOpType

    nc = bacc.Bacc("TRN2", target_bir_lowering=False, debug=False, num_devices=8)

    xq_d = nc.dram_tensor("xq", [65, L], bf16, kind="ExternalInput").ap()
    xk_d = nc.dram_tensor("xk", [65, L], bf16, kind="ExternalInput").ap()
    xv_d = nc.dram_tensor("xv", [65, L], bf16, kind="ExternalInput").ap()
    # blobA: [65, 292] = wq[0:128] | wk[128:256] | wv[256:292]
    wa_d = nc.dram_tensor("wa", [65, 292], bf16, kind="ExternalInput").ap()
    # blobB: [128, 193] = wo[0:64] | sel[64:192] | bo2 col [0:64, 192]
    wb_d = nc.dram_tensor("wb", [128, 193], bf16, kind="ExternalInput").ap()
    tri_d = nc.dram_tensor("tri", [128, 512], bf16, kind="ExternalInput").ap()
    y_d = nc.dram_tensor("y", [64, L], f32, kind="ExternalOutput").ap()

    from contextlib import ExitStack

    # ---- build-time ACT/DVE exp assignment (greedy balance of ns totals) ----
    def act_cost(free):
        return (free + 352) / 1.2

    def dve_cost(free):
        return (free + 120) / 0.96

    assign = {}
    act_tot = [1283.0 + 430.0]  # table load + warm
    dve_tot = [0.0]

    def plan_group(g):
        if g % 2 == 0:
            # proj copies split across both engines (k/q halves + v)
            act_tot[0] += 2 * act_cost(512) + act_cost(288)
            dve_tot[0] += 2 * dve_cost(512) + dve_cost(288)
        n_s = 4 * g + 4 if causal else NT
        if g == 0:
            dve_tot[0] += 484 + 594  # av memset + dummy-warm memset
        for t in range(n_s):
            col_off = max(0, (t - 4 * g) * 128) if causal else 0
            npr = 512 - col_off
            if causal and t >= 4 * g:
                dve_tot[0] += (151 + 256) / 0.96  # tri mask (bf16 2x)
            for p_ in range(2):
                fa, fd = act_cost(2 * npr), dve_cost(2 * npr)
                if act_tot[0] + fa <= dve_tot[0] + fd:
                    assign[(g, t, p_)] = "ACT"
                    act_tot[0] += fa
                else:
                    assign[(g, t, p_)] = "DVE"
                    dve_tot[0] += fd
        # epilogue: av_sb copy on ACT; rcp + cast + avn + y bias on DVE
        act_tot[0] += act_cost(512)
        dve_tot[0] += (58 + 512) / 0.96 + (58 + 256) / 0.96 \
            + (151 + 512) / 0.96 + dve_cost(512)

    for g in range(NG):
        plan_group(g)

    with tile.TileContext(nc) as tc, ExitStack() as es:
        singles = es.enter_context(tc.tile_pool(name="singles", bufs=1))
        persist = es.enter_context(tc.tile_pool(name="persist", bufs=1))
        sc_ps = es.enter_context(tc.tile_pool(name="sc_ps", bufs=3, space="PSUM"))
        av_ps_pool = es.enter_context(tc.tile_pool(name="av_ps", bufs=1, space="PSUM"))
        misc_ps = es.enter_context(tc.tile_pool(name="misc_ps", bufs=1, space="PSUM"))
        attn_pool = es.enter_context(tc.tile_pool(name="attn", bufs=8))
        small = es.enter_context(tc.tile_pool(name="small", bufs=2))

        # ---- load inputs ----
        xq = singles.tile([65, L], bf16, tag="xq")
        xk = singles.tile([65, L], bf16, tag="xk")
        xv = singles.tile([65, L], bf16, tag="xv")
        wa = singles.tile([65, 292], bf16, tag="wa")
        wb = singles.tile([128, 193], bf16, tag="wb")
        tri = singles.tile([128, 512], bf16, tag="tri")

        wq = wa[:, 0:128]
        wk = wa[:, 128:256]
        wv = wa[:, 256:292]
        wo = wb[:, 0:64]
        sel = wb[:, 64:192]

        # spread input DMA: weights on scalar (idle early), xk/xv on sync,
        # xq on gpsimd, so the wk+xk0 / xq0 critical chains run in parallel
        s0, s1 = slice(0, 1024), slice(1024, 2048)
        nc.scalar.dma_start(out=wa[:], in_=wa_d[:])
        nc.sync.dma_start(out=xk[:, s0], in_=xk_d[:, s0])
        nc.gpsimd.dma_start(out=xq[:, s0], in_=xq_d[:, s0])
        nc.scalar.dma_start(out=tri[:], in_=tri_d[:])
        nc.sync.dma_start(out=xv[:, s0], in_=xv_d[:, s0])
        nc.gpsimd.dma_start(out=wb[:], in_=wb_d[:])
        nc.sync.dma_start(out=xk[:, s1], in_=xk_d[:, s1])
        nc.gpsimd.dma_start(out=xq[:, s1], in_=xq_d[:, s1])
        nc.sync.dma_start(out=xv[:, s1], in_=xv_d[:, s1])

        # preload the ACT exp table during the rest of the DMA
        warm = singles.tile([1, 292], f32, tag="warm")
        nc.scalar.activation(out=warm[:], in_=wa[:1, :], func=AF.Exp)
        bo_col = singles.tile([64, 1], f32, tag="bo_col")

        # ---- persistent projected tensors ----
        qT = persist.tile([128, L], bf16, tag="qT")
        kT = persist.tile([128, L], bf16, tag="kT")
        v_aug = persist.tile([128, 16 * 36], bf16, tag="v_aug")

        def proj_piece(jh, piece):
            # one fine projection piece (1 matmul + 1 copy) staged through the
            # misc PSUM bank so the sc pool stays dedicated to span tiles;
            # copies alternate ACT/DVE so no engine eats a burst
            act = piece in ("k0", "q1", "v0")
            pj = misc_ps.tile([128, 512], f32, tag="misc")
            if piece[0] in "kq":
                m = int(piece[1])
                w_, dst, src_x = (wk, kT, xk) if piece[0] == "k" else (wq, qT, xq)
                s2 = slice(1024 * jh + 512 * m, 1024 * jh + 512 * (m + 1))
                nc.tensor.matmul(out=pj[:], lhsT=w_, rhs=src_x[:, s2],
                                 start=True, stop=True)
                if jh == 0 and m == 0:
                    # startup critical path: split across both engines
                    cut = 128 if piece[0] == "k" else 256
                    nc.scalar.copy(dst[:, 0:cut], pj[:, 0:cut])
                    nc.vector.tensor_copy(dst[:, cut:512], pj[:, cut:512])
                elif act:
                    nc.scalar.copy(dst[:, s2], pj[:])
                else:
                    nc.vector.tensor_copy(dst[:, s2], pj[:])
            else:
                half = int(piece[1])  # "v0" / "v1"
                for tt in range(4):
                    t = 8 * jh + 4 * half + tt
                    nc.tensor.matmul(out=pj[:, 128 * tt:128 * tt + 36],
                                     lhsT=xv[:, 128 * t:128 * (t + 1)], rhs=wv,
                                     start=True, stop=True)
                dst = v_aug.rearrange("p (c n) -> p c n", n=36)[
                    :, 8 * jh + 4 * half:8 * jh + 4 * half + 4, :]
                src = pj.rearrange("p (c n) -> p c n", n=128)[:, :, :36]
                if act:
                    nc.scalar.copy(dst, src)
                else:
                    nc.vector.tensor_copy(dst, src)

        # ---- attention ----
        def make_epilogue(g, av):
            # normalize + output projection; nbl column blocks pipelined so
            # the last group's chain hides behind its trailing spans
            def run(nbl):
                w = 512 // nbl
                y_sb = small.tile([64, 512], f32, tag="y_sb")
                av_sb = small.tile([128, 512], f32, tag="av_sb")
                rcp = small.tile([128, 512], f32, tag="rcp")
                rcp_bf = small.tile([128, 512], bf16, tag="rcp_bf")
                avn = small.tile([128, 512], bf16, tag="avn")
                for b_ in range(nbl):
                    cs = slice(w * b_, w * (b_ + 1))
                    if b_ % 2 == 0:
                        nc.scalar.copy(av_sb[:, cs], av[:, cs])
                    else:
                        nc.vector.tensor_copy(av_sb[:, cs], av[:, cs])
                    nc.vector.reciprocal_approx_fast(out=rcp[:, cs], in_=av_sb[:, cs])
                    nc.vector.tensor_copy(rcp_bf[:, cs], rcp[:, cs])
                    if nbl > 1:
                        # last group: spans are done, so stage each block
                        # through a fresh sc-pool tile — blocks then pipeline
                        # across banks instead of serializing through misc
                        pj = sc_ps.tile([128, 1024], f32, tag="sc")
                        bc_ap = pj[:, 0:w]
                        yp_ap = pj[0:64, 512:512 + w]
                    else:
                        bc = misc_ps.tile([128, 512], f32, tag="misc")
                        bc_ap = bc[:, cs]
                    nc.tensor.matmul(out=bc_ap, lhsT=sel, rhs=rcp_bf[:, cs],
                                     start=True, stop=True)
                    nc.vector.tensor_mul(avn[:, cs], av_sb[:, cs], bc_ap)
                    if nbl == 1:
                        yp = misc_ps.tile([64, 512], f32, tag="misc")
                        yp_ap = yp[:, cs]
                    nc.tensor.matmul(out=yp_ap, lhsT=wo, rhs=avn[:, cs],
                                     start=True, stop=True)
                    nc.vector.tensor_scalar(out=y_sb[:, cs], in0=yp_ap,
                                            scalar1=bo_col[:],
                                            scalar2=None, op0=OP.add)
                nc.sync.dma_start(out=y_d[:, 512 * g:512 * (g + 1)], in_=y_sb[:])
            return run

        # PE warm-up: dummy matmuls fill the input-DMA wait so HAM reaches
        # 2.4 GHz before the first real matmul (results overwritten/ignored);
        # alternating output addresses avoid WAW drain serialization.
        # The memset must be the FIRST vector-queue op (no DMA dependency)
        # or the dummies start late behind DMA-gated copies.
        dummy_in = singles.tile([128, 256], bf16, tag="dummy_in")
        nc.vector.memset(dummy_in[:], 1.0)
        dmy = misc_ps.tile([128, 512], f32, tag="misc")
        for i in range(14):
            nc.tensor.matmul(out=dmy[:, 256 * (i % 2):256 * (i % 2) + 256],
                             lhsT=dummy_in[:, :128],
                             rhs=dummy_in[:], start=True, stop=True)

        pending_epi = None
        for g in range(NG):
            if g == 0:
                proj_piece(0, "k0")
                proj_piece(0, "q0")
            n_s = 4 * g + 4 if causal else NT
            # 1.0 (not 0) so junk rows stay finite through reciprocal below;
            # rows 9..31 of each 32-group persist across groups (PE never
            # writes them), so one memset at kernel start suffices
            av = av_ps_pool.tile([128, 512], f32, tag="av")
            if g == 0:
                nc.vector.memset(av[:], 1.0)
            pending_av = []

            def emit_av(t, col_off, attnT, g=g, n_s=n_s, av=av):
                for c in range(4):
                    nc.tensor.matmul(
                        out=av[32 * c:32 * c + 9, col_off:512],
                        lhsT=v_aug[:, 36 * t + 9 * c:36 * t + 9 * c + 9],
                        rhs=attnT[:, 512 * c + col_off:512 * (c + 1)],
                        start=(t == 0), stop=(t == n_s - 1),
                        tile_position=(0, 32 * c))

            for t in range(n_s):
                if len(pending_av) > 4:
                    emit_av(*pending_av.pop(0))
                col_off = max(0, (t - 4 * g) * 128) if causal else 0
                npr = 512 - col_off
                scA = sc_ps.tile([128, 1024], f32, tag="sc")
                scB = sc_ps.tile([128, 1024], f32, tag="sc")
                for r in range(4):
                    sc = scA if r < 2 else scB
                    c0 = 512 * (r % 2) + col_off
                    nc.tensor.matmul(
                        out=sc[:, c0:c0 + npr],
                        lhsT=kT[32 * r:32 * r + 8, 128 * t:128 * (t + 1)],
                        rhs=qT[32 * r:32 * r + 8, 512 * g + col_off:512 * (g + 1)],
                        start=True, stop=True, tile_position=(32 * r, 0))
                attnT = attn_pool.tile([128, 2048], bf16, tag="attnT")
                a4 = attnT.rearrange("p (h n) -> p h n", h=4)
                for p_ in range(2):
                    sc = scA if p_ == 0 else scB
                    s2 = sc.rearrange("p (h n) -> p h n", h=2)
                    if assign[(g, t, p_)] == "ACT":
                        nc.scalar.activation(
                            out=a4[:, 2 * p_:2 * p_ + 2, col_off:512],
                            in_=s2[:, :, col_off:512],
                            func=AF.Exp)
                    else:
                        nc.vector.tensor_scalar(
                            out=a4[:, 2 * p_:2 * p_ + 2, col_off:512].bitcast(i16),
                            in0=s2[:, :, col_off:512],
                            scalar1=EXP_A, scalar2=EXP_B,
                            op0=OP.mult, op1=OP.add)
                if causal and t >= 4 * g:
                    t4 = tri.rearrange("p (h n) -> p h n", h=4)
                    nc.vector.tensor_mul(
                        a4[:, :, col_off:col_off + 128],
                        a4[:, :, col_off:col_off + 128],
                        t4[:, :, :])
                pending_av.append((t, col_off, attnT))
                # previous group's deferred epilogue + next projections slot in
                # behind this group's first spans (engines have slack here)
                if t == 1 and pending_epi is not None:
                    pending_epi(1)
                    pending_epi = None
                if g == 0 and 0 <= t <= 2:
                    proj_piece(0, ("v0", "q1", "k1")[t])
                if g == 0 and t == 1:
                    nc.vector.tensor_copy(bo_col[:], wb[0:64, 192:193])
                if g == 1 and t == 0:
                    proj_piece(0, "v1")
                if g == 1 and 2 <= t <= 7:
                    proj_piece(1, ("q0", "k0", "q1", "k1", "v0", "v1")[t - 2])
            for args in pending_av:
                emit_av(*args)
            pending_epi = make_epilogue(g, av)
        # last group's epilogue: 4 pipelined column blocks
        pending_epi(4 if causal else 1)

    nc.compile()
    return nc


def _prep_inputs(queries, keys, values, Wq, bq, Wk, bk, Wv, bv, Wo, bo):
    """Build the 8 per-core input maps (host-side layout/sharding only)."""
    import ml_dtypes
    bf = ml_dtypes.bfloat16
    ones = np.ones((1, L), np.float32)

    def aug_t(x_b):  # [L, 64] -> [65, L] bf16
        return np.ascontiguousarray(np.vstack([x_b.T, ones]).astype(bf))

    # padded projection weights per quad: col 32r+d <- head (4Q+r) dim d
    def w_pad(W, b, quad, scale=1.0):
        out = np.zeros((65, 128), np.float32)
        for r in range(4):
            ch = 8 * (4 * quad + r)
            out[:64, 32 * r:32 * r + 8] = W[ch:ch + 8, :].T * scale
            out[64, 32 * r:32 * r + 8] = b[ch:ch + 8] * scale
        return out

    def wv_aug(quad):  # [65, 36]: col 9c+e <- head (4Q+c) dim e; col 9c+8 = e64
        out = np.zeros((65, 36), np.float32)
        for c in range(4):
            ch = 8 * (4 * quad + c)
            out[:64, 9 * c:9 * c + 8] = Wv[ch:ch + 8, :].T
            out[64, 9 * c:9 * c + 8] = bv[ch:ch + 8]
            out[64, 9 * c + 8] = 1.0
        return out

    def wo_pad(quad):  # [128, 64]: row 32c+d -> Wo[:, 8(4Q+c)+d]
        out = np.zeros((128, 64), np.float32)
        for c in range(4):
            ch = 8 * (4 * quad + c)
            out[32 * c:32 * c + 8, :] = Wo[:, ch:ch + 8].T
        return out

    tri = (np.arange(128)[:, None] <= np.arange(128)[None, :]).astype(np.float32)
    tri4 = np.ascontiguousarray(np.tile(tri, (1, 4)).astype(bf))
    sel = np.zeros((128, 128), np.float32)
    for c in range(4):
        sel[32 * c + 8, 32 * c:32 * c + 9] = 1.0
    bo2 = (bo.astype(np.float32) / 2.0).reshape(64)

    wa_cache, wb_cache = {}, {}
    for qd in range(2):
        wa = np.zeros((65, 292), np.float32)
        wa[:, 0:128] = w_pad(Wq, bq, qd, scale=float(SCALE))
        wa[:, 128:256] = w_pad(Wk, bk, qd)
        wa[:, 256:292] = wv_aug(qd)
        wb = np.zeros((128, 193), np.float32)
        wb[:, 0:64] = wo_pad(qd)
        wb[:, 64:192] = sel
        wb[0:64, 192] = bo2
        wa_cache[qd] = np.ascontiguousarray(wa.astype(bf))
        wb_cache[qd] = np.ascontiguousarray(wb.astype(bf))

    in_maps = []
    for c in range(8):
        b, qd = c // 2, c % 2
        in_maps.append(dict(
            xq=aug_t(np.asarray(queries[b])),
            xk=aug_t(np.asarray(keys[b])),
            xv=aug_t(np.asarray(values[b])),
            wa=wa_cache[qd], wb=wb_cache[qd], tri=tri4,
        ))
    return in_maps


def _install_trace_hook():
    import contextlib
    import ctypes
    import types

    name = "antenv.axon_hooks"
    if name in sys.modules:
        return
    so_path = "/opt/axon/libaxon_pjrt.so"
    if not os.path.exists(so_path):
        return
    lib = ctypes.CDLL(so_path)
    if not hasattr(lib, "axon_start_nrt_profile"):
        return
    lib.axon_start_nrt_profile.argtypes = [ctypes.POINTER(ctypes.c_int64), ctypes.c_size_t]
    lib.axon_start_nrt_profile.restype = ctypes.c_int64
    lib.axon_stop_nrt_profile.argtypes = [ctypes.c_char_p]
    lib.axon_stop_nrt_profile.restype = ctypes.c_int64

    @contextlib.contextmanager
    def _hook(output_dir, device_ids):
        import jax
        jax.devices()
        if device_ids:
            ids = (ctypes.c_int64 * len(device_ids))(*device_ids)
            rc = lib.axon_start_nrt_profile(ids, len(device_ids))
        else:
            rc = lib.axon_start_nrt_profile(None, 0)
        if rc != 0:
            raise RuntimeError(f"axon_start_nrt_profile rc={rc}")
        try:
            yield
        finally:
            n = lib.axon_stop_nrt_profile(str(output_dir).encode())
            print(f"profile: {n} file(s) in {output_dir}", file=sys.stderr)

    mod = types.ModuleType(name)
    mod._hook = _hook
    mod.set_axon_ntff_profile_hook = lambda h: setattr(mod, "_hook", h)
    mod.get_axon_ntff_profile_hook = lambda: mod._hook
    sys.modules[name] = mod


def kernel(queries, keys, values, attention_mask, Wq, bq, Wk, bk, Wv, bv, Wo, bo):
    global LAST_EXEC_NS
    from concourse.bass_utils import run_bass_kernel_spmd

    causal = bool(int(np.asarray(attention_mask)))
    if causal not in _CACHE:
        _CACHE[causal] = _build(causal)
    nc = _CACHE[causal]

    in_maps = _prep_inputs(queries, keys, values, Wq, bq, Wk, bk, Wv, bv, Wo, bo)

    trace = os.environ.get("KERNEL_TRACE", "") == "1"
    kwargs = {}
    if trace:
        _install_trace_hook()
        kwargs = dict(trace=True, tmpdir=os.environ.get("KERNEL_TRACE_DIR") or None)
    res = run_bass_kernel_spmd(nc, in_maps, core_ids=list(range(8)), **kwargs)
    LAST_EXEC_NS = res.exec_time_ns

    out = np.empty((B, L, D), np.float32)
    for b in range(B):
        out[b] = (res.results[2 * b]["y"] + res.results[2 * b + 1]["y"]).T
    return out
